# revision 31
# baseline (speedup 1.0000x reference)
"""FFT-encoded attention (nn_Attention_78065325572136) on 8 Trainium2 cores.

Math (per batch b, reproducing the reference exactly):
  feat = [Re rfft(x)/C, -Im rfft(x)/C]  ->  folded into weights on host:
     Wq = E @ wq.T * hd^-0.5,  Wk = E @ wk.T,  Wv = E @ wv.T   (E = DFT-real matrix)
     Wp = wproj.T @ D,  bp = bproj @ D                         (D = irfft matrix)
  so the whole module becomes matmuls + softmax:
     M_q = x_b @ Wq, M_k = x_b @ Wk, M_v = x_b @ Wv            [N, C]
     per head h (the reference's quirky reshape):
        q_h = M_q[h*64:(h+1)*64, :].reshape(N, 64)   (same for k_h, v_h)
        o[:, h*64:(h+1)*64] = softmax(q_h @ k_h.T) @ v_h
     out_b = o @ Wp + bp

Device kernel (one batch per NeuronCore, pure data-parallel SPMD, no
collectives): everything in bf16 matmuls with fp32 PSUM accumulation.
Queries and keys are processed in a permuted order nt = s*64 + r
(original n = r*16 + s) which turns the quirky head reshape into
hardware-friendly strided DMAs; the output DMA un-permutes rows.

Softmax has no max-subtraction (scores here are mathematically tiny),
row sums come from a ones-vector matmul on the tensor engine, and the
division is applied to the (transposed) attention output via a GPSIMD
partition-broadcast of the reciprocal row.
"""

import os
import sys

import numpy as np

for _p in ("/opt/trn_rl_repo", "/root/.axon_site/_ro/trn_rl_repo"):
    if os.path.isdir(_p) and _p not in sys.path:
        sys.path.append(_p)

import ml_dtypes

import concourse.bass as bass
import concourse.mybir as mybir
import concourse.tile as tile
from concourse import library_config
from concourse.bass_utils import run_bass_kernel_spmd

BF16 = ml_dtypes.bfloat16
B, N, C, H = 8, 1024, 1024, 16
HD = C // H            # 64
F = C // 2 + 1         # 513
NCORES = 8

# ---------------------------------------------------------------------------
# Walrus workaround: the staged neuronxcc rejects CTRL_NO_STRUCT instructions
# (the Tile kernel-tail Drain) carrying more than one SyncWait. Split excess
# waits onto dedicated no-fuse InstNoOp carriers on the same engine queue.
# ---------------------------------------------------------------------------
_MAX_WAITS = 1


def _split_waits_in_module(nc):
    for f in nc.m.functions:
        for bb in f.blocks:
            out, changed = [], False
            for inst in list(bb.instructions):
                si = inst.sync_info
                if si is not None and len(si.on_wait) > _MAX_WAITS:
                    waits = list(si.on_wait)
                    keep, excess = waits[-_MAX_WAITS:], waits[:-_MAX_WAITS]
                    for i in range(0, len(excess), _MAX_WAITS):
                        nop = mybir.InstNoOp(
                            name=f"I-{nc.next_id()}-waitcarrier",
                            engine=inst.engine,
                            bass_nofuse=True,
                            sync_info=mybir.SyncInfo(
                                on_wait=excess[i : i + _MAX_WAITS], on_update=[]
                            ),
                        )
                        nc.register_instruction(nop, overwrite=True)
                        out.append(nop)
                        changed = True
                    inst.sync_info = mybir.SyncInfo(
                        on_wait=keep, on_update=list(si.on_update)
                    )
                out.append(inst)
            if changed:
                bb.instructions = out


_orig_drain_and_barrier = tile.TileContext._drain_and_barrier


def _patched_drain_and_barrier(self, tick_clock, wait_clock):
    _orig_drain_and_barrier(self, tick_clock, wait_clock)
    _split_waits_in_module(self.nc)


def _install_tile_patch():
    tile.TileContext._drain_and_barrier = _patched_drain_and_barrier


_install_tile_patch()

# ---------------------------------------------------------------------------
# Host-side weight folding (DFT matrices are input-independent constants).
# ---------------------------------------------------------------------------


def _dft_matrices():
    c = np.arange(C)[:, None].astype(np.float64)
    j = np.arange(F)[None, :].astype(np.float64)
    ang = 2.0 * np.pi * c * j / C
    E = np.concatenate([np.cos(ang) / C, np.sin(ang) / C], axis=1)  # [C, 2F]
    Fh = C // 2
    jj = np.arange(Fh)[:, None].astype(np.float64)
    cc = np.arange(C)[None, :].astype(np.float64)
    ang2 = 2.0 * np.pi * jj * cc / C
    w = np.full((Fh, 1), 2.0)
    w[0, 0] = 1.0
    D = np.concatenate([w * np.cos(ang2), w * np.sin(ang2)], axis=0)  # [C, C]
    return E.astype(np.float32), D.astype(np.float32)


_E, _D = _dft_matrices()

# ---------------------------------------------------------------------------
# Device kernel builder.
# ---------------------------------------------------------------------------

F32 = mybir.dt.float32
BF = mybir.dt.bfloat16


def build_nc():
    nc = bass.Bass()
    xT = nc.declare_dram_parameter("xT", [C, N], BF, isOutput=False)
    wq = nc.declare_dram_parameter("wq", [C, C], BF, isOutput=False)
    wk = nc.declare_dram_parameter("wk", [C, C], BF, isOutput=False)
    wv = nc.declare_dram_parameter("wv", [C, C], BF, isOutput=False)
    wp = nc.declare_dram_parameter("wp", [C, C], BF, isOutput=False)
    bp = nc.declare_dram_parameter("bp", [1, C], BF, isOutput=False)
    out = nc.declare_dram_parameter("out", [N, C], F32, isOutput=True)

    # Intermediates, pair-major so attention for early head-pairs can start
    # while later projection chunks are still running (Tile DRAM deps are
    # whole-tensor): qt/kt hold M_q.T / M_k.T columns regrouped per pair
    # ([4 pairs, C, 128]); vv is M_v split into one tensor per pair.
    qts = [nc.dram_tensor(f"qt{half}", [4, C, 128], BF) for half in range(2)]
    kts = [nc.dram_tensor(f"kt{half}", [4, C, 128], BF) for half in range(2)]
    vvs = [nc.dram_tensor(f"vv{mi}", [128, C], BF) for mi in range(8)]

    Exp = mybir.ActivationFunctionType.Exp
    Copy = mybir.ActivationFunctionType.Copy

    with tile.TileContext(nc) as tc:
        with (
            tc.tile_pool(name="const", bufs=1) as consts,
            tc.tile_pool(name="win", bufs=1) as win,
            tc.tile_pool(name="winx", bufs=1) as winx,
            tc.tile_pool(name="evict", bufs=4) as evict,
            tc.tile_pool(name="attin", bufs=2) as attin,
            tc.tile_pool(name="ptiles", bufs=2) as ptiles,
            tc.tile_pool(name="stiles", bufs=2) as stiles,
            tc.tile_pool(name="otiles", bufs=1) as otiles,
        ):
            # ---- load inputs into SBUF ----
            def load_rows(src, n_tiles=8, width=None, dtype=BF, pool=win,
                          eng=None):
                w_ = width or src.shape[1]
                ts = []
                for k in range(n_tiles):
                    t = pool.tile([128, w_], dtype, tag=f"ld_{src.tensor.name}_{k}")
                    (eng or nc.sync).dma_start(t[:], src[k * 128 : (k + 1) * 128, :])
                    ts.append(t)
                return ts

            # interleave x/wq so the first projection's deps land first
            xts, wqs = [], []
            for k in range(8):
                t = winx.tile([128, N], BF, tag=f"ld_xT_{k}", name="xt")
                nc.sync.dma_start(t[:], xT[k * 128 : (k + 1) * 128, :])
                xts.append(t)
                t = winx.tile([128, C], BF, tag=f"ld_wq_{k}", name="wqt")
                nc.sync.dma_start(t[:], wq[k * 128 : (k + 1) * 128, :])
                wqs.append(t)
            wks = load_rows(wk[:], pool=winx)
            wvs = load_rows(wv[:], pool=winx, eng=nc.gpsimd)
            wps = load_rows(wp[:], eng=nc.gpsimd)

            bpt = consts.tile([1, C], BF)
            nc.sync.dma_start(bpt[:], bp[:])
            ones_row = consts.tile([1, 128], BF)
            nc.gpsimd.memset(ones_row[:], 1.0)
            ones128 = consts.tile([128, 128], BF)
            nc.gpsimd.memset(ones128[:], 1.0)

            psum_ctx = tc.tile_pool(name="psatt", bufs=2, space="PSUM")
            psum = psum_ctx.__enter__()

            # ---- phase 1: projections ----
            # qt = Wq.T @ xT  (lhsT = wq chunk, rhs = xT chunk), written
            # pair-major; kt likewise; vv = x @ Wv written per pair-row.
            # Emission order puts everything head-pairs 0-3 need first.
            def proj_tile(wt, dst_kind, mi, ni, x_is_lhs):
                ps = psum.tile([128, 512], F32, tag="proj", name="ps", bufs=1)
                for ki in range(8):
                    lhsT = (xts[ki] if x_is_lhs else wt[ki])[
                        :, mi * 128 : (mi + 1) * 128
                    ]
                    rhs = (wt[ki] if x_is_lhs else xts[ki])[
                        :, ni * 512 : (ni + 1) * 512
                    ]
                    nc.tensor.matmul(
                        ps[:], lhsT, rhs, start=(ki == 0), stop=(ki == 7)
                    )
                sb = evict.tile([128, 512], BF, tag="projev", name="sb")
                nc.scalar.activation(sb[:], ps[:], Copy)
                if x_is_lhs:
                    # vv_mi rows, columns ni*512:
                    nc.sync.dma_start(
                        vvs[mi][:, ni * 512 : (ni + 1) * 512], sb[:]
                    )
                else:
                    # qt/kt half `ni`: [4, C, 128]; sbuf free n = j*128 + nl
                    dst = dst_kind[ni][:].rearrange("j c l -> c j l")
                    nc.sync.dma_start(
                        dst[mi * 128 : (mi + 1) * 128], sb[:]
                    )

            def proj_half(half):
                for mi in range(8):
                    proj_tile(wqs, qts, mi, half, False)
                for mi in range(8):
                    proj_tile(wks, kts, mi, half, False)
                for mi in range(4 * half, 4 * half + 4):
                    for ni in range(2):
                        proj_tile(wvs, None, mi, ni, True)

            oT_tiles = []

            # ---- phase 2: attention, one head-pair at a time ----
            def attention_pair(hp):
                half, j = hp // 4, hp % 4
                qt_v = qts[half][:].rearrange(
                    "j (s d) (h r) -> j h d s r", s=16, d=64, h=2, r=64
                )
                kt_v = kts[half][:].rearrange(
                    "j (s d) (h r) -> j h d s r", s=16, d=64, h=2, r=64
                )
                vv_v = vvs[hp][:].rearrange(
                    "(h r) (sc s2 d) -> h s2 r sc d", h=2, r=64, sc=8, s2=2, d=64
                )
                qtp = attin.tile([128, N], BF, tag="qtp")
                ktp = attin.tile([128, N], BF, tag="ktp")
                vp = attin.tile([128, N], BF, tag="vp")
                for h in range(2):
                    hs = slice(h * 64, (h + 1) * 64)
                    nc.sync.dma_start(qtp[hs, :], qt_v[j, h])
                    nc.sync.dma_start(ktp[hs, :], kt_v[j, h])
                    for s2 in range(2):
                        vslice = vp[s2 * 64 : (s2 + 1) * 64, :].rearrange(
                            "p (sc h d) -> p sc h d", sc=8, h=2, d=64
                        )[:, :, h, :]
                        nc.sync.dma_start(vslice, vv_v[h, s2])

                oTp = otiles.tile([128, N], BF, tag=f"oT{hp}")
                oT_tiles.append(oTp)

                for ni in range(2):
                    n1s = slice(ni * 512, (ni + 1) * 512)
                    PA = ptiles.tile([128, 8 * 512], BF, tag="PA")
                    PB = ptiles.tile([128, 8 * 512], BF, tag="PB")
                    # scores + exp: scoreT[nt2, nt1] row-tiled head pair.
                    # Two nt2-chunks share one 2-bank psum tile so each exp
                    # covers [128, 1024] (halves ACT per-op overhead).
                    for c2p in range(4):
                        psa = psum.tile([128, 1024], F32, tag="scA", bufs=1)
                        psb = psum.tile([128, 1024], F32, tag="scB", bufs=1)
                        for k in range(2):
                            c2 = 2 * c2p + k
                            c2s = slice(c2 * 128, (c2 + 1) * 128)
                            ph = slice(k * 512, (k + 1) * 512)
                            nc.tensor.matmul(
                                psa[:, ph], ktp[0:64, c2s], qtp[0:64, n1s],
                                start=True, stop=True, tile_position=(0, 0),
                            )
                            nc.tensor.matmul(
                                psb[:, ph], ktp[64:128, c2s], qtp[64:128, n1s],
                                start=True, stop=True, tile_position=(64, 0),
                            )
                        nc.scalar.activation(
                            PA[:, c2p * 1024 : (c2p + 1) * 1024], psa[:], Exp
                        )
                        nc.scalar.activation(
                            PB[:, c2p * 1024 : (c2p + 1) * 1024], psb[:], Exp
                        )
                    # row sums: chunk-reduce on DVE, then ones-matmul on PE,
                    # reciprocal, partition-broadcast
                    rbs = []
                    for P_, tagc in ((PA, "A"), (PB, "B")):
                        # pairwise add-tree over the 8 chunks (bf16 4x DVE mode)
                        S2 = stiles.tile([128, 2048], BF, tag=f"S2{tagc}", bufs=1)
                        nc.vector.tensor_add(S2[:], P_[:, 0:2048], P_[:, 2048:4096])
                        S4 = stiles.tile([128, 1024], BF, tag=f"S4{tagc}", bufs=1)
                        nc.vector.tensor_add(S4[:], S2[:, 0:1024], S2[:, 1024:2048])
                        S = stiles.tile([128, 512], BF, tag=f"S{tagc}", bufs=1)
                        nc.vector.tensor_add(S[:], S4[:, 0:512], S4[:, 512:1024])
                        # all-ones lhsT: one matmul yields the row-sum
                        # already broadcast across all 128 psum partitions
                        rps = psum.tile([128, 512], F32, tag="psR", bufs=1,
                                        name="rps")
                        nc.tensor.matmul(
                            rps[:], ones128[:], S[:], start=True, stop=True
                        )
                        rb = stiles.tile([128, 512], BF, tag=f"rb{tagc}",
                                         bufs=1)
                        with nc.allow_low_precision(
                            reason="softmax 1/rowsum in bf16; fine vs the "
                            "2e-2 gate"
                        ):
                            nc.vector.reciprocal(rb[:], rps[:])
                        rbs.append(rb)
                    # o.T = v.T @ P, col-tiled head pair, accumulate over nt2
                    psoA = psum.tile([128, 512], F32, tag="psOA", bufs=1)
                    psoB = psum.tile([128, 512], F32, tag="psOB", bufs=1)
                    for c2 in range(8):
                        nc.tensor.matmul(
                            psoA[0:64, :],
                            vp[:, c2 * 128 : c2 * 128 + 64],
                            PA[:, c2 * 512 : (c2 + 1) * 512],
                            start=(c2 == 0), stop=(c2 == 7), tile_position=(0, 0),
                        )
                        nc.tensor.matmul(
                            psoB[64:128, :],
                            vp[:, c2 * 128 + 64 : c2 * 128 + 128],
                            PB[:, c2 * 512 : (c2 + 1) * 512],
                            start=(c2 == 0), stop=(c2 == 7), tile_position=(0, 64),
                        )
                    nc.vector.tensor_mul(oTp[0:64, n1s], psoA[0:64, :], rbs[0][0:64, :])
                    nc.vector.tensor_mul(
                        oTp[64:128, n1s], psoB[64:128, :], rbs[1][64:128, :]
                    )

            proj_half(0)
            for hp in range(4):
                attention_pair(hp)
            proj_half(1)
            for hp in range(4, 8):
                attention_pair(hp)

            # partial of the output projection over head-pairs 0-3: emitted
            # after all attention work so it has the lowest priority -- PE
            # filler during the ACT-bound tail of pairs 4-7 -- and shortens
            # the serial fin tail to a 4-deep accumulation.
            parts = {}
            for mi in range(8):
                for ni in range(2):
                    psp = psum.tile([128, 512], F32, tag="proj", bufs=1,
                                    name="psp")
                    for kp in range(4):
                        nc.tensor.matmul(
                            psp[:],
                            oT_tiles[kp][:, mi * 128 : (mi + 1) * 128],
                            wps[kp][:, ni * 512 : (ni + 1) * 512],
                            start=(kp == 0), stop=(kp == 3),
                        )
                    pt = stiles.tile([128, 512], BF, tag=f"part{mi}_{ni}",
                                     bufs=1, name="pt")
                    nc.vector.tensor_copy(pt[:], psp[:])
                    parts[(mi, ni)] = pt

            # ---- phase 3: out = o @ Wp + bp, rows written un-permuted ----
            winx_released = True
            psum_ctx.__exit__(None, None, None)
            psfin_ctx = tc.tile_pool(name="psfin", bufs=2, space="PSUM")
            psfin = psfin_ctx.__enter__()
            out_v = out[:].rearrange("(r sm s2) c -> sm s2 r c", r=64, sm=8, s2=2)
            for mi in range(8):
                for ni in range(2):
                    ps = psfin.tile([128, 512], F32, tag="fin")
                    nc.tensor.matmul(
                        ps[:], ones_row[:], bpt[0:1, ni * 512 : (ni + 1) * 512],
                        start=True, stop=False,
                    )
                    for kp in range(4, 8):
                        nc.tensor.matmul(
                            ps[:],
                            oT_tiles[kp][:, mi * 128 : (mi + 1) * 128],
                            wps[kp][:, ni * 512 : (ni + 1) * 512],
                            start=False, stop=(kp == 7),
                        )
                    ob = evict.tile([128, 512], F32, tag="outev")
                    nc.vector.tensor_add(ob[:], ps[:], parts[(mi, ni)][:])
                    for s2 in range(2):
                        nc.sync.dma_start(
                            out_v[mi, s2][:, ni * 512 : (ni + 1) * 512],
                            ob[s2 * 64 : (s2 + 1) * 64, :],
                        )
            psfin_ctx.__exit__(None, None, None)

    return nc


# ---------------------------------------------------------------------------
# Host wrapper.
# ---------------------------------------------------------------------------

_NC_CACHE = None


def _get_nc():
    global _NC_CACHE
    if _NC_CACHE is None:
        _NC_CACHE = build_nc()
    return _NC_CACHE


def host_inputs(x, wq, wk, wv, wproj, bproj):
    """Fold DFT matrices into the weights; per-core input maps."""
    scale = float(HD) ** -0.5
    Wq = (_E @ wq.T.astype(np.float32) * scale).astype(BF16)
    Wk = (_E @ wk.T.astype(np.float32)).astype(BF16)
    Wv = (_E @ wv.T.astype(np.float32)).astype(BF16)
    Wp = (wproj.T.astype(np.float32) @ _D).astype(BF16)
    bpD = (bproj.astype(np.float32) @ _D).astype(BF16).reshape(1, C)
    in_maps = []
    for b in range(B):
        xTb = np.ascontiguousarray(x[b].T).astype(BF16)
        in_maps.append(
            {"xT": xTb, "wq": Wq, "wk": Wk, "wv": Wv, "wp": Wp, "bp": bpD}
        )
    return in_maps


def kernel(x, wq, wk, wv, wproj, bproj):
    x = np.asarray(x, dtype=np.float32)
    in_maps = host_inputs(
        x,
        np.asarray(wq, np.float32),
        np.asarray(wk, np.float32),
        np.asarray(wv, np.float32),
        np.asarray(wproj, np.float32),
        np.asarray(bproj, np.float32),
    )
    nc = _get_nc()
    res = run_bass_kernel_spmd(nc, in_maps, list(range(NCORES)))
    out = np.stack([np.asarray(res.results[i]["out"]) for i in range(NCORES)])
    return out.astype(np.float32)


# revision 51
# speedup vs baseline: 1.0719x; 1.0719x over previous
"""FFT-encoded attention (nn_Attention_78065325572136) on 8 Trainium2 cores.

Math (per batch b, reproducing the reference exactly):
  feat = [Re rfft(x)/C, -Im rfft(x)/C]  ->  folded into weights on host:
     Wq = E @ wq.T * hd^-0.5,  Wk = E @ wk.T,  Wv = E @ wv.T   (E = DFT-real matrix)
     Wp = wproj.T @ D,  bp = bproj @ D                         (D = irfft matrix)
  so the whole module becomes matmuls + softmax:
     M_q = x_b @ Wq, M_k = x_b @ Wk, M_v = x_b @ Wv            [N, C]
     per head h (the reference's quirky reshape):
        q_h = M_q[h*64:(h+1)*64, :].reshape(N, 64)   (same for k_h, v_h)
        o[:, h*64:(h+1)*64] = softmax(q_h @ k_h.T) @ v_h
     out_b = o @ Wp + bp

Device kernel (one batch per NeuronCore, pure data-parallel SPMD, no
collectives): everything in bf16 matmuls with fp32 PSUM accumulation.
Queries and keys are processed in a permuted order nt = s*64 + r
(original n = r*16 + s) which turns the quirky head reshape into
hardware-friendly strided DMAs; the output DMA un-permutes rows.

Softmax has no max-subtraction (scores here are mathematically tiny).
Row sums come from an all-ones matmul on the tensor engine, which also
broadcasts them across all psum partitions; the division is applied to
the (transposed) attention output after a bf16 reciprocal.
"""

import os
import sys

import numpy as np

for _p in ("/opt/trn_rl_repo", "/root/.axon_site/_ro/trn_rl_repo"):
    if os.path.isdir(_p) and _p not in sys.path:
        sys.path.append(_p)

import ml_dtypes

import concourse.bass as bass
import concourse.mybir as mybir
import concourse.tile as tile
from concourse.bass_utils import run_bass_kernel_spmd

BF16 = ml_dtypes.bfloat16
B, N, C, H = 8, 1024, 1024, 16
HD = C // H            # 64
F = C // 2 + 1         # 513
NCORES = 8

# ---------------------------------------------------------------------------
# Walrus workaround: the staged neuronxcc rejects CTRL_NO_STRUCT instructions
# (the Tile kernel-tail Drain) carrying more than one SyncWait. Split excess
# waits onto dedicated no-fuse InstNoOp carriers on the same engine queue.
# ---------------------------------------------------------------------------
_MAX_WAITS = 1


def _split_waits_in_module(nc):
    for f in nc.m.functions:
        for bb in f.blocks:
            out, changed = [], False
            for inst in list(bb.instructions):
                si = inst.sync_info
                if si is not None and len(si.on_wait) > _MAX_WAITS:
                    waits = list(si.on_wait)
                    keep, excess = waits[-_MAX_WAITS:], waits[:-_MAX_WAITS]
                    for i in range(0, len(excess), _MAX_WAITS):
                        nop = mybir.InstNoOp(
                            name=f"I-{nc.next_id()}-waitcarrier",
                            engine=inst.engine,
                            bass_nofuse=True,
                            sync_info=mybir.SyncInfo(
                                on_wait=excess[i : i + _MAX_WAITS], on_update=[]
                            ),
                        )
                        nc.register_instruction(nop, overwrite=True)
                        out.append(nop)
                        changed = True
                    inst.sync_info = mybir.SyncInfo(
                        on_wait=keep, on_update=list(si.on_update)
                    )
                out.append(inst)
            if changed:
                bb.instructions = out


_orig_drain_and_barrier = tile.TileContext._drain_and_barrier


def _patched_drain_and_barrier(self, tick_clock, wait_clock):
    _orig_drain_and_barrier(self, tick_clock, wait_clock)
    _split_waits_in_module(self.nc)


def _install_tile_patch():
    tile.TileContext._drain_and_barrier = _patched_drain_and_barrier


_install_tile_patch()

# ---------------------------------------------------------------------------
# Host-side weight folding (DFT matrices are input-independent constants).
# ---------------------------------------------------------------------------


def _dft_matrices():
    c = np.arange(C)[:, None].astype(np.float64)
    j = np.arange(F)[None, :].astype(np.float64)
    ang = 2.0 * np.pi * c * j / C
    E = np.concatenate([np.cos(ang) / C, np.sin(ang) / C], axis=1)  # [C, 2F]
    Fh = C // 2
    jj = np.arange(Fh)[:, None].astype(np.float64)
    cc = np.arange(C)[None, :].astype(np.float64)
    ang2 = 2.0 * np.pi * jj * cc / C
    w = np.full((Fh, 1), 2.0)
    w[0, 0] = 1.0
    D = np.concatenate([w * np.cos(ang2), w * np.sin(ang2)], axis=0)  # [C, C]
    return E.astype(np.float32), D.astype(np.float32)


_E, _D = _dft_matrices()

# ---------------------------------------------------------------------------
# Device kernel builder.
# ---------------------------------------------------------------------------

F32 = mybir.dt.float32
BF = mybir.dt.bfloat16


def build_nc():
    nc = bass.Bass()
    xT = nc.declare_dram_parameter("xT", [C, N], BF, isOutput=False)
    wq = nc.declare_dram_parameter("wq", [C, C], BF, isOutput=False)
    wk = nc.declare_dram_parameter("wk", [C, C], BF, isOutput=False)
    wv = nc.declare_dram_parameter("wv", [C, C], BF, isOutput=False)
    wp = nc.declare_dram_parameter("wp", [C, C], BF, isOutput=False)
    bp = nc.declare_dram_parameter("bp", [1, C], BF, isOutput=False)
    out = nc.declare_dram_parameter("out", [N, C], F32, isOutput=True)

    # Intermediates, pair-major so attention for early head-pairs can start
    # while later projection chunks are still running (Tile DRAM deps are
    # whole-tensor): qt/kt hold M_q.T / M_k.T columns regrouped per pair
    # ([4 pairs, C, 128]); vv is M_v split into one tensor per pair.
    qts = [nc.dram_tensor(f"qt{hp}", [C, 128], BF) for hp in range(8)]
    kts = [nc.dram_tensor(f"kt{hp}", [C, 128], BF) for hp in range(8)]
    vvs = [nc.dram_tensor(f"vv{mi}", [128, C], BF) for mi in range(8)]

    Exp = mybir.ActivationFunctionType.Exp
    Copy = mybir.ActivationFunctionType.Copy

    with tile.TileContext(nc) as tc:
        with (
            tc.tile_pool(name="const", bufs=1) as consts,
            tc.tile_pool(name="win", bufs=1) as win,
            tc.tile_pool(name="winx", bufs=1) as winx,
            tc.tile_pool(name="evict", bufs=4) as evict,
            tc.tile_pool(name="attin", bufs=2) as attin,
            tc.tile_pool(name="ptiles", bufs=2) as ptiles,
            tc.tile_pool(name="stiles", bufs=2) as stiles,
            tc.tile_pool(name="otiles", bufs=1) as otiles,
        ):
            # ---- load inputs into SBUF ----
            def load_rows(src, n_tiles=8, width=None, dtype=BF, pool=win,
                          eng=None):
                w_ = width or src.shape[1]
                ts = []
                for k in range(n_tiles):
                    t = pool.tile([128, w_], dtype, tag=f"ld_{src.tensor.name}_{k}")
                    (eng or nc.sync).dma_start(t[:], src[k * 128 : (k + 1) * 128, :])
                    ts.append(t)
                return ts

            # interleave x/wq so the first projection's deps land first
            xts, wqs = [], []
            for k in range(8):
                t = winx.tile([128, N], BF, tag=f"ld_xT_{k}", name="xt")
                (nc.sync if k % 2 == 0 else nc.gpsimd).dma_start(
                    t[:], xT[k * 128 : (k + 1) * 128, :]
                )
                xts.append(t)
                t = winx.tile([128, C], BF, tag=f"ld_wq_{k}", name="wqt")
                nc.scalar.dma_start(t[:], wq[k * 128 : (k + 1) * 128, :])
                wqs.append(t)
            wks = load_rows(wk[:], pool=winx)
            wvs = load_rows(wv[:], pool=winx, eng=nc.gpsimd)
            wps = load_rows(wp[:], eng=nc.gpsimd)

            bpt = consts.tile([1, C], BF)
            nc.sync.dma_start(bpt[:], bp[:])
            ones_row = consts.tile([1, 128], BF)
            nc.gpsimd.memset(ones_row[:], 1.0)
            ones128 = consts.tile([128, 128], BF)
            nc.gpsimd.memset(ones128[:], 1.0)

            psum_ctx = tc.tile_pool(name="psatt", bufs=2, space="PSUM")
            psum = psum_ctx.__enter__()

            # ---- phase 1: projections, emitted per head-pair ----
            # qt_hp = (Wq.T @ xT)[:, hp*128:+128]  as [C, 128]  (N=128 tiles)
            # kt_hp likewise; vv_hp = (x @ Wv)[hp*128:+128, :]  as [128, C]
            def proj_pair(hp):
                for wt, dst in ((wqs, qts[hp]), (wks, kts[hp])):
                    for mi in range(8):
                        ps = psum.tile([128, 128], F32, tag="proj", name="ps",
                                       bufs=1)
                        for ki in range(8):
                            nc.tensor.matmul(
                                ps[:],
                                wt[ki][:, mi * 128 : (mi + 1) * 128],
                                xts[ki][:, hp * 128 : (hp + 1) * 128],
                                start=(ki == 0), stop=(ki == 7),
                            )
                        sb = evict.tile([128, 128], BF, tag="projev", name="sb")
                        nc.scalar.activation(sb[:], ps[:], Copy)
                        nc.sync.dma_start(
                            dst[mi * 128 : (mi + 1) * 128, :], sb[:]
                        )
                for ni in range(2):
                    ps = psum.tile([128, 512], F32, tag="proj", name="ps",
                                   bufs=1)
                    for ki in range(8):
                        nc.tensor.matmul(
                            ps[:],
                            xts[ki][:, hp * 128 : (hp + 1) * 128],
                            wvs[ki][:, ni * 512 : (ni + 1) * 512],
                            start=(ki == 0), stop=(ki == 7),
                        )
                    sb = evict.tile([128, 512], BF, tag="projev", name="sb")
                    nc.scalar.activation(sb[:], ps[:], Copy)
                    nc.sync.dma_start(
                        vvs[hp][:, ni * 512 : (ni + 1) * 512], sb[:]
                    )

            oT_tiles = []

            # ---- phase 2: attention, one head-pair at a time ----
            def attention_pair(hp):
                qt_v = qts[hp][:].rearrange(
                    "(s d) (h r) -> h d s r", s=16, d=64, h=2, r=64
                )
                kt_v = kts[hp][:].rearrange(
                    "(s d) (h r) -> h d s r", s=16, d=64, h=2, r=64
                )
                vv_v = vvs[hp][:].rearrange(
                    "(h r) (sc s2 d) -> h s2 r sc d", h=2, r=64, sc=8, s2=2, d=64
                )
                qtp = attin.tile([128, N], BF, tag="qtp")
                ktp = attin.tile([128, N], BF, tag="ktp")
                vp = attin.tile([128, N], BF, tag="vp")
                for h in range(2):
                    hs = slice(h * 64, (h + 1) * 64)
                    nc.sync.dma_start(qtp[hs, :], qt_v[h])
                    nc.sync.dma_start(ktp[hs, :], kt_v[h])
                    for s2 in range(2):
                        vslice = vp[s2 * 64 : (s2 + 1) * 64, :].rearrange(
                            "p (sc h d) -> p sc h d", sc=8, h=2, d=64
                        )[:, :, h, :]
                        nc.gpsimd.dma_start(vslice, vv_v[h, s2])

                oTp = otiles.tile([128, N], BF, tag=f"oT{hp}")
                oT_tiles.append(oTp)

                for ni in range(2):
                    n1s = slice(ni * 512, (ni + 1) * 512)
                    PA = ptiles.tile([128, 8 * 512], BF, tag="PA")
                    PB = ptiles.tile([128, 8 * 512], BF, tag="PB")
                    # scores + exp: scoreT[nt2, nt1] row-tiled head pair.
                    # Two nt2-chunks share one 2-bank psum tile so each exp
                    # covers [128, 1024] (halves ACT per-op overhead).
                    for c2p in range(4):
                        psa = psum.tile([128, 1024], F32, tag="scA", bufs=1)
                        psb = psum.tile([128, 1024], F32, tag="scB", bufs=1)
                        for k in range(2):
                            c2 = 2 * c2p + k
                            c2s = slice(c2 * 128, (c2 + 1) * 128)
                            ph = slice(k * 512, (k + 1) * 512)
                            nc.tensor.matmul(
                                psa[:, ph], ktp[0:64, c2s], qtp[0:64, n1s],
                                start=True, stop=True, tile_position=(0, 0),
                            )
                            nc.tensor.matmul(
                                psb[:, ph], ktp[64:128, c2s], qtp[64:128, n1s],
                                start=True, stop=True, tile_position=(64, 0),
                            )
                        nc.scalar.activation(
                            PA[:, c2p * 1024 : (c2p + 1) * 1024], psa[:], Exp
                        )
                        nc.scalar.activation(
                            PB[:, c2p * 1024 : (c2p + 1) * 1024], psb[:], Exp
                        )
                    # row sums: chunk-reduce on DVE, then ones-matmul on PE,
                    # reciprocal, partition-broadcast
                    rbs = []
                    for P_, tagc in ((PA, "A"), (PB, "B")):
                        # pairwise add-tree over the 8 chunks (bf16 4x DVE mode)
                        S2 = stiles.tile([128, 2048], BF, tag=f"S2{tagc}", bufs=1)
                        nc.vector.tensor_add(S2[:], P_[:, 0:2048], P_[:, 2048:4096])
                        S4 = stiles.tile([128, 1024], BF, tag=f"S4{tagc}", bufs=1)
                        nc.vector.tensor_add(S4[:], S2[:, 0:1024], S2[:, 1024:2048])
                        S = stiles.tile([128, 512], BF, tag=f"S{tagc}", bufs=1)
                        nc.vector.tensor_add(S[:], S4[:, 0:512], S4[:, 512:1024])
                        # all-ones lhsT: one matmul yields the row-sum
                        # already broadcast across all 128 psum partitions
                        rps = psum.tile([128, 512], F32, tag="psR", bufs=1,
                                        name="rps")
                        nc.tensor.matmul(
                            rps[:], ones128[:], S[:], start=True, stop=True
                        )
                        rb = stiles.tile([128, 512], BF, tag=f"rb{tagc}",
                                         bufs=1)
                        with nc.allow_low_precision(
                            reason="softmax 1/rowsum in bf16; fine vs the "
                            "2e-2 gate"
                        ):
                            nc.vector.reciprocal(rb[:], rps[:])
                        rbs.append(rb)
                    # o.T = v.T @ P, col-tiled head pair, accumulate over nt2
                    psoA = psum.tile([128, 512], F32, tag="psOA", bufs=1)
                    psoB = psum.tile([128, 512], F32, tag="psOB", bufs=1)
                    for c2 in range(8):
                        nc.tensor.matmul(
                            psoA[0:64, :],
                            vp[:, c2 * 128 : c2 * 128 + 64],
                            PA[:, c2 * 512 : (c2 + 1) * 512],
                            start=(c2 == 0), stop=(c2 == 7), tile_position=(0, 0),
                        )
                        nc.tensor.matmul(
                            psoB[64:128, :],
                            vp[:, c2 * 128 + 64 : c2 * 128 + 128],
                            PB[:, c2 * 512 : (c2 + 1) * 512],
                            start=(c2 == 0), stop=(c2 == 7), tile_position=(0, 64),
                        )
                    nc.vector.tensor_mul(oTp[0:64, n1s], psoA[0:64, :], rbs[0][0:64, :])
                    nc.vector.tensor_mul(
                        oTp[64:128, n1s], psoB[64:128, :], rbs[1][64:128, :]
                    )

            proj_pair(0)
            proj_pair(1)
            for hp in range(8):
                attention_pair(hp)
                if hp + 2 < 8:
                    proj_pair(hp + 2)

            # ---- phase 3: out = o @ Wp + bp, rows written un-permuted ----
            winx_released = True
            psum_ctx.__exit__(None, None, None)
            psfin_ctx = tc.tile_pool(name="psfin", bufs=4, space="PSUM")
            psfin = psfin_ctx.__enter__()
            out_v = out[:].rearrange("(r sm s2) c -> sm s2 r c", r=64, sm=8, s2=2)
            for mi in range(8):
                for ni in range(2):
                    ps = psfin.tile([128, 512], F32, tag="fin")
                    nc.tensor.matmul(
                        ps[:], ones_row[:], bpt[0:1, ni * 512 : (ni + 1) * 512],
                        start=True, stop=False,
                    )
                    for kp in range(8):
                        nc.tensor.matmul(
                            ps[:],
                            oT_tiles[kp][:, mi * 128 : (mi + 1) * 128],
                            wps[kp][:, ni * 512 : (ni + 1) * 512],
                            start=False, stop=(kp == 7),
                        )
                    ob = evict.tile([128, 512], F32, tag="outev")
                    nc.vector.tensor_copy(ob[:], ps[:])
                    for s2 in range(2):
                        nc.sync.dma_start(
                            out_v[mi, s2][:, ni * 512 : (ni + 1) * 512],
                            ob[s2 * 64 : (s2 + 1) * 64, :],
                        )
            psfin_ctx.__exit__(None, None, None)

    return nc


# ---------------------------------------------------------------------------
# Host wrapper.
# ---------------------------------------------------------------------------

_NC_CACHE = None


def _get_nc():
    global _NC_CACHE
    if _NC_CACHE is None:
        _NC_CACHE = build_nc()
    return _NC_CACHE


def host_inputs(x, wq, wk, wv, wproj, bproj):
    """Fold DFT matrices into the weights; per-core input maps."""
    scale = float(HD) ** -0.5
    Wq = (_E @ wq.T.astype(np.float32) * scale).astype(BF16)
    Wk = (_E @ wk.T.astype(np.float32)).astype(BF16)
    Wv = (_E @ wv.T.astype(np.float32)).astype(BF16)
    Wp = (wproj.T.astype(np.float32) @ _D).astype(BF16)
    bpD = (bproj.astype(np.float32) @ _D).astype(BF16).reshape(1, C)
    in_maps = []
    for b in range(B):
        xTb = np.ascontiguousarray(x[b].T).astype(BF16)
        in_maps.append(
            {"xT": xTb, "wq": Wq, "wk": Wk, "wv": Wv, "wp": Wp, "bp": bpD}
        )
    return in_maps


def kernel(x, wq, wk, wv, wproj, bproj):
    x = np.asarray(x, dtype=np.float32)
    in_maps = host_inputs(
        x,
        np.asarray(wq, np.float32),
        np.asarray(wk, np.float32),
        np.asarray(wv, np.float32),
        np.asarray(wproj, np.float32),
        np.asarray(bproj, np.float32),
    )
    nc = _get_nc()
    res = run_bass_kernel_spmd(nc, in_maps, list(range(NCORES)))
    out = np.stack([np.asarray(res.results[i]["out"]) for i in range(NCORES)])
    return out.astype(np.float32)


# revision 56
# speedup vs baseline: 1.0778x; 1.0055x over previous
"""FFT-encoded attention (nn_Attention_78065325572136) on 8 Trainium2 cores.

Math (per batch b, reproducing the reference exactly):
  feat = [Re rfft(x)/C, -Im rfft(x)/C]  ->  folded into weights on host:
     Wq = E @ wq.T * hd^-0.5,  Wk = E @ wk.T,  Wv = E @ wv.T   (E = DFT-real matrix)
     Wp = wproj.T @ D,  bp = bproj @ D                         (D = irfft matrix)
  so the whole module becomes matmuls + softmax:
     M_q = x_b @ Wq, M_k = x_b @ Wk, M_v = x_b @ Wv            [N, C]
     per head h (the reference's quirky reshape):
        q_h = M_q[h*64:(h+1)*64, :].reshape(N, 64)   (same for k_h, v_h)
        o[:, h*64:(h+1)*64] = softmax(q_h @ k_h.T) @ v_h
     out_b = o @ Wp + bp

Device kernel (one batch per NeuronCore, pure data-parallel SPMD, no
collectives): everything in bf16 matmuls with fp32 PSUM accumulation.
Queries and keys are processed in a permuted order nt = s*64 + r
(original n = r*16 + s) which turns the quirky head reshape into
hardware-friendly strided DMAs; the output DMA un-permutes rows.

Softmax has no max-subtraction (scores here are mathematically tiny).
Row sums come from an all-ones matmul on the tensor engine, which also
broadcasts them across all psum partitions; the division is applied to
the (transposed) attention output after a bf16 reciprocal.
"""

import os
import sys

import numpy as np

for _p in ("/opt/trn_rl_repo", "/root/.axon_site/_ro/trn_rl_repo"):
    if os.path.isdir(_p) and _p not in sys.path:
        sys.path.append(_p)

import ml_dtypes

import concourse.bass as bass
import concourse.mybir as mybir
import concourse.tile as tile
from concourse.bass_utils import run_bass_kernel_spmd

BF16 = ml_dtypes.bfloat16
B, N, C, H = 8, 1024, 1024, 16
HD = C // H            # 64
F = C // 2 + 1         # 513
NCORES = 8

# ---------------------------------------------------------------------------
# Walrus workaround: the staged neuronxcc rejects CTRL_NO_STRUCT instructions
# (the Tile kernel-tail Drain) carrying more than one SyncWait. Split excess
# waits onto dedicated no-fuse InstNoOp carriers on the same engine queue.
# ---------------------------------------------------------------------------
_MAX_WAITS = 1


def _split_waits_in_module(nc):
    for f in nc.m.functions:
        for bb in f.blocks:
            out, changed = [], False
            for inst in list(bb.instructions):
                si = inst.sync_info
                if si is not None and len(si.on_wait) > _MAX_WAITS:
                    waits = list(si.on_wait)
                    keep, excess = waits[-_MAX_WAITS:], waits[:-_MAX_WAITS]
                    for i in range(0, len(excess), _MAX_WAITS):
                        nop = mybir.InstNoOp(
                            name=f"I-{nc.next_id()}-waitcarrier",
                            engine=inst.engine,
                            bass_nofuse=True,
                            sync_info=mybir.SyncInfo(
                                on_wait=excess[i : i + _MAX_WAITS], on_update=[]
                            ),
                        )
                        nc.register_instruction(nop, overwrite=True)
                        out.append(nop)
                        changed = True
                    inst.sync_info = mybir.SyncInfo(
                        on_wait=keep, on_update=list(si.on_update)
                    )
                out.append(inst)
            if changed:
                bb.instructions = out


_orig_drain_and_barrier = tile.TileContext._drain_and_barrier


def _patched_drain_and_barrier(self, tick_clock, wait_clock):
    _orig_drain_and_barrier(self, tick_clock, wait_clock)
    _split_waits_in_module(self.nc)


def _install_tile_patch():
    tile.TileContext._drain_and_barrier = _patched_drain_and_barrier


_install_tile_patch()

# ---------------------------------------------------------------------------
# Host-side weight folding (DFT matrices are input-independent constants).
# ---------------------------------------------------------------------------


def _dft_matrices():
    c = np.arange(C)[:, None].astype(np.float64)
    j = np.arange(F)[None, :].astype(np.float64)
    ang = 2.0 * np.pi * c * j / C
    E = np.concatenate([np.cos(ang) / C, np.sin(ang) / C], axis=1)  # [C, 2F]
    Fh = C // 2
    jj = np.arange(Fh)[:, None].astype(np.float64)
    cc = np.arange(C)[None, :].astype(np.float64)
    ang2 = 2.0 * np.pi * jj * cc / C
    w = np.full((Fh, 1), 2.0)
    w[0, 0] = 1.0
    D = np.concatenate([w * np.cos(ang2), w * np.sin(ang2)], axis=0)  # [C, C]
    return E.astype(np.float32), D.astype(np.float32)


_E, _D = _dft_matrices()

# ---------------------------------------------------------------------------
# Device kernel builder.
# ---------------------------------------------------------------------------

F32 = mybir.dt.float32
BF = mybir.dt.bfloat16


def build_nc():
    nc = bass.Bass()
    xT = nc.declare_dram_parameter("xT", [C, N], BF, isOutput=False)
    wq = nc.declare_dram_parameter("wq", [C, C], BF, isOutput=False)
    wk = nc.declare_dram_parameter("wk", [C, C], BF, isOutput=False)
    wv = nc.declare_dram_parameter("wv", [C, C], BF, isOutput=False)
    wp = nc.declare_dram_parameter("wp", [C, C], BF, isOutput=False)
    bp = nc.declare_dram_parameter("bp", [1, C], BF, isOutput=False)
    out = nc.declare_dram_parameter("out", [N, C], F32, isOutput=True)

    # Intermediates, one tensor per head-pair so attention for a pair can
    # start as soon as its own projection columns land (Tile DRAM deps are
    # whole-tensor): qt/kt hold M_q.T / M_k.T column blocks [C, 128]; vv
    # holds M_v row blocks [128, C].
    qts = [nc.dram_tensor(f"qt{hp}", [C, 128], BF) for hp in range(8)]
    kts = [nc.dram_tensor(f"kt{hp}", [C, 128], BF) for hp in range(8)]
    vvs = [nc.dram_tensor(f"vv{mi}", [128, C], BF) for mi in range(8)]

    Exp = mybir.ActivationFunctionType.Exp
    Copy = mybir.ActivationFunctionType.Copy

    with tile.TileContext(nc) as tc:
        with (
            tc.tile_pool(name="const", bufs=1) as consts,
            tc.tile_pool(name="win", bufs=1) as win,
            tc.tile_pool(name="winx", bufs=1) as winx,
            tc.tile_pool(name="evict", bufs=4) as evict,
            tc.tile_pool(name="attin", bufs=2) as attin,
            tc.tile_pool(name="ptiles", bufs=2) as ptiles,
            tc.tile_pool(name="stiles", bufs=2) as stiles,
            tc.tile_pool(name="otiles", bufs=1) as otiles,
        ):
            # ---- load inputs into SBUF ----
            def load_rows(src, n_tiles=8, width=None, dtype=BF, pool=win,
                          eng=None):
                w_ = width or src.shape[1]
                ts = []
                for k in range(n_tiles):
                    t = pool.tile([128, w_], dtype, tag=f"ld_{src.tensor.name}_{k}")
                    (eng or nc.sync).dma_start(t[:], src[k * 128 : (k + 1) * 128, :])
                    ts.append(t)
                return ts

            # interleave x/wq so the first projection's deps land first
            xts, wqs = [], []
            for k in range(8):
                t = winx.tile([128, N], BF, tag=f"ld_xT_{k}", name="xt")
                (nc.sync if k % 2 == 0 else nc.gpsimd).dma_start(
                    t[:], xT[k * 128 : (k + 1) * 128, :]
                )
                xts.append(t)
                t = winx.tile([128, C], BF, tag=f"ld_wq_{k}", name="wqt")
                nc.scalar.dma_start(t[:], wq[k * 128 : (k + 1) * 128, :])
                wqs.append(t)
            wks = load_rows(wk[:], pool=winx)
            wvs = load_rows(wv[:], pool=winx, eng=nc.gpsimd)
            wps = load_rows(wp[:], eng=nc.gpsimd)

            bpt = consts.tile([1, C], BF)
            nc.sync.dma_start(bpt[:], bp[:])
            ones_row = consts.tile([1, 128], BF)
            nc.gpsimd.memset(ones_row[:], 1.0)
            ones128 = consts.tile([128, 128], BF)
            nc.gpsimd.memset(ones128[:], 1.0)

            psum_ctx = tc.tile_pool(name="psatt", bufs=2, space="PSUM")
            psum = psum_ctx.__enter__()

            # ---- phase 1: projections, emitted per head-pair ----
            # qt_hp = (Wq.T @ xT)[:, hp*128:+128]  as [C, 128]  (N=128 tiles)
            # kt_hp likewise; vv_hp = (x @ Wv)[hp*128:+128, :]  as [128, C]
            def proj_pair(hp):
                for wt, dst in ((wqs, qts[hp]), (wks, kts[hp])):
                    for mi in range(8):
                        ps = psum.tile([128, 128], F32, tag="proj", name="ps",
                                       bufs=1)
                        for ki in range(8):
                            nc.tensor.matmul(
                                ps[:],
                                wt[ki][:, mi * 128 : (mi + 1) * 128],
                                xts[ki][:, hp * 128 : (hp + 1) * 128],
                                start=(ki == 0), stop=(ki == 7),
                            )
                        sb = evict.tile([128, 128], BF, tag="projev", name="sb")
                        nc.vector.tensor_copy(sb[:], ps[:])
                        nc.sync.dma_start(
                            dst[mi * 128 : (mi + 1) * 128, :], sb[:]
                        )
                for ni in range(2):
                    ps = psum.tile([128, 512], F32, tag="proj", name="ps",
                                   bufs=1)
                    for ki in range(8):
                        nc.tensor.matmul(
                            ps[:],
                            xts[ki][:, hp * 128 : (hp + 1) * 128],
                            wvs[ki][:, ni * 512 : (ni + 1) * 512],
                            start=(ki == 0), stop=(ki == 7),
                        )
                    sb = evict.tile([128, 512], BF, tag="projev", name="sb")
                    nc.scalar.activation(sb[:], ps[:], Copy)
                    nc.sync.dma_start(
                        vvs[hp][:, ni * 512 : (ni + 1) * 512], sb[:]
                    )

            oT_tiles = []

            # ---- phase 2: attention, one head-pair at a time ----
            def attention_pair(hp):
                qt_v = qts[hp][:].rearrange(
                    "(s d) (h r) -> h d s r", s=16, d=64, h=2, r=64
                )
                kt_v = kts[hp][:].rearrange(
                    "(s d) (h r) -> h d s r", s=16, d=64, h=2, r=64
                )
                vv_v = vvs[hp][:].rearrange(
                    "(h r) (sc s2 d) -> h s2 r sc d", h=2, r=64, sc=8, s2=2, d=64
                )
                qtp = attin.tile([128, N], BF, tag="qtp")
                ktp = attin.tile([128, N], BF, tag="ktp")
                vp = attin.tile([128, N], BF, tag="vp")
                for h in range(2):
                    hs = slice(h * 64, (h + 1) * 64)
                    nc.sync.dma_start(qtp[hs, :], qt_v[h])
                    nc.sync.dma_start(ktp[hs, :], kt_v[h])
                    for s2 in range(2):
                        vslice = vp[s2 * 64 : (s2 + 1) * 64, :].rearrange(
                            "p (sc h d) -> p sc h d", sc=8, h=2, d=64
                        )[:, :, h, :]
                        nc.gpsimd.dma_start(vslice, vv_v[h, s2])

                oTp = otiles.tile([128, N], BF, tag=f"oT{hp}")
                oT_tiles.append(oTp)

                for ni in range(2):
                    n1s = slice(ni * 512, (ni + 1) * 512)
                    PA = ptiles.tile([128, 8 * 512], BF, tag="PA")
                    PB = ptiles.tile([128, 8 * 512], BF, tag="PB")
                    # scores + exp: scoreT[nt2, nt1] row-tiled head pair.
                    # Two nt2-chunks share one 2-bank psum tile so each exp
                    # covers [128, 1024] (halves ACT per-op overhead).
                    for c2p in range(4):
                        psa = psum.tile([128, 1024], F32, tag="scA", bufs=1)
                        psb = psum.tile([128, 1024], F32, tag="scB", bufs=1)
                        for k in range(2):
                            c2 = 2 * c2p + k
                            c2s = slice(c2 * 128, (c2 + 1) * 128)
                            ph = slice(k * 512, (k + 1) * 512)
                            nc.tensor.matmul(
                                psa[:, ph], ktp[0:64, c2s], qtp[0:64, n1s],
                                start=True, stop=True, tile_position=(0, 0),
                            )
                            nc.tensor.matmul(
                                psb[:, ph], ktp[64:128, c2s], qtp[64:128, n1s],
                                start=True, stop=True, tile_position=(64, 0),
                            )
                        nc.scalar.activation(
                            PA[:, c2p * 1024 : (c2p + 1) * 1024], psa[:], Exp
                        )
                        nc.scalar.activation(
                            PB[:, c2p * 1024 : (c2p + 1) * 1024], psb[:], Exp
                        )
                    # row sums: chunk-reduce on DVE, then ones-matmul on PE,
                    # reciprocal, partition-broadcast
                    rbs = []
                    for P_, tagc in ((PA, "A"), (PB, "B")):
                        # pairwise add-tree over the 8 chunks (bf16 4x DVE mode)
                        S2 = stiles.tile([128, 2048], BF, tag=f"S2{tagc}", bufs=1)
                        nc.vector.tensor_add(S2[:], P_[:, 0:2048], P_[:, 2048:4096])
                        S4 = stiles.tile([128, 1024], BF, tag=f"S4{tagc}", bufs=1)
                        nc.vector.tensor_add(S4[:], S2[:, 0:1024], S2[:, 1024:2048])
                        S = stiles.tile([128, 512], BF, tag=f"S{tagc}", bufs=1)
                        nc.vector.tensor_add(S[:], S4[:, 0:512], S4[:, 512:1024])
                        # all-ones lhsT: one matmul yields the row-sum
                        # already broadcast across all 128 psum partitions
                        rps = psum.tile([128, 512], F32, tag="psR", bufs=1,
                                        name="rps")
                        nc.tensor.matmul(
                            rps[:], ones128[:], S[:], start=True, stop=True
                        )
                        rb = stiles.tile([128, 512], BF, tag=f"rb{tagc}",
                                         bufs=1)
                        with nc.allow_low_precision(
                            reason="softmax 1/rowsum in bf16; fine vs the "
                            "2e-2 gate"
                        ):
                            nc.vector.reciprocal(rb[:], rps[:])
                        rbs.append(rb)
                    # o.T = v.T @ P, col-tiled head pair, accumulate over nt2
                    psoA = psum.tile([128, 512], F32, tag="psOA", bufs=1)
                    psoB = psum.tile([128, 512], F32, tag="psOB", bufs=1)
                    for c2 in range(8):
                        nc.tensor.matmul(
                            psoA[0:64, :],
                            vp[:, c2 * 128 : c2 * 128 + 64],
                            PA[:, c2 * 512 : (c2 + 1) * 512],
                            start=(c2 == 0), stop=(c2 == 7), tile_position=(0, 0),
                        )
                        nc.tensor.matmul(
                            psoB[64:128, :],
                            vp[:, c2 * 128 + 64 : c2 * 128 + 128],
                            PB[:, c2 * 512 : (c2 + 1) * 512],
                            start=(c2 == 0), stop=(c2 == 7), tile_position=(0, 64),
                        )
                    nc.vector.tensor_mul(oTp[0:64, n1s], psoA[0:64, :], rbs[0][0:64, :])
                    nc.vector.tensor_mul(
                        oTp[64:128, n1s], psoB[64:128, :], rbs[1][64:128, :]
                    )

            proj_pair(0)
            proj_pair(1)
            for hp in range(8):
                attention_pair(hp)
                if hp + 2 < 8:
                    proj_pair(hp + 2)

            # ---- phase 3: out = o @ Wp + bp, rows written un-permuted ----
            winx_released = True
            psum_ctx.__exit__(None, None, None)
            psfin_ctx = tc.tile_pool(name="psfin", bufs=4, space="PSUM")
            psfin = psfin_ctx.__enter__()
            out_v = out[:].rearrange("(r sm s2) c -> sm s2 r c", r=64, sm=8, s2=2)
            for mi in range(8):
                for ni in range(2):
                    ps = psfin.tile([128, 512], F32, tag="fin")
                    nc.tensor.matmul(
                        ps[:], ones_row[:], bpt[0:1, ni * 512 : (ni + 1) * 512],
                        start=True, stop=False,
                    )
                    for kp in range(8):
                        nc.tensor.matmul(
                            ps[:],
                            oT_tiles[kp][:, mi * 128 : (mi + 1) * 128],
                            wps[kp][:, ni * 512 : (ni + 1) * 512],
                            start=False, stop=(kp == 7),
                        )
                    ob = evict.tile([128, 512], F32, tag="outev")
                    nc.scalar.activation(ob[:], ps[:], Copy)
                    for s2 in range(2):
                        nc.sync.dma_start(
                            out_v[mi, s2][:, ni * 512 : (ni + 1) * 512],
                            ob[s2 * 64 : (s2 + 1) * 64, :],
                        )
            psfin_ctx.__exit__(None, None, None)

    return nc


# ---------------------------------------------------------------------------
# Host wrapper.
# ---------------------------------------------------------------------------

_NC_CACHE = None


def _get_nc():
    global _NC_CACHE
    if _NC_CACHE is None:
        _NC_CACHE = build_nc()
    return _NC_CACHE


def host_inputs(x, wq, wk, wv, wproj, bproj):
    """Fold DFT matrices into the weights; per-core input maps."""
    scale = float(HD) ** -0.5
    Wq = (_E @ wq.T.astype(np.float32) * scale).astype(BF16)
    Wk = (_E @ wk.T.astype(np.float32)).astype(BF16)
    Wv = (_E @ wv.T.astype(np.float32)).astype(BF16)
    Wp = (wproj.T.astype(np.float32) @ _D).astype(BF16)
    bpD = (bproj.astype(np.float32) @ _D).astype(BF16).reshape(1, C)
    in_maps = []
    for b in range(B):
        xTb = np.ascontiguousarray(x[b].T).astype(BF16)
        in_maps.append(
            {"xT": xTb, "wq": Wq, "wk": Wk, "wv": Wv, "wp": Wp, "bp": bpD}
        )
    return in_maps


def kernel(x, wq, wk, wv, wproj, bproj):
    x = np.asarray(x, dtype=np.float32)
    in_maps = host_inputs(
        x,
        np.asarray(wq, np.float32),
        np.asarray(wk, np.float32),
        np.asarray(wv, np.float32),
        np.asarray(wproj, np.float32),
        np.asarray(bproj, np.float32),
    )
    nc = _get_nc()
    res = run_bass_kernel_spmd(nc, in_maps, list(range(NCORES)))
    out = np.stack([np.asarray(res.results[i]["out"]) for i in range(NCORES)])
    return out.astype(np.float32)


# revision 57
# speedup vs baseline: 1.0919x; 1.0131x over previous
"""FFT-encoded attention (nn_Attention_78065325572136) on 8 Trainium2 cores.

Math (per batch b, reproducing the reference exactly):
  feat = [Re rfft(x)/C, -Im rfft(x)/C]  ->  folded into weights on host:
     Wq = E @ wq.T * hd^-0.5,  Wk = E @ wk.T,  Wv = E @ wv.T   (E = DFT-real matrix)
     Wp = wproj.T @ D,  bp = bproj @ D                         (D = irfft matrix)
  so the whole module becomes matmuls + softmax:
     M_q = x_b @ Wq, M_k = x_b @ Wk, M_v = x_b @ Wv            [N, C]
     per head h (the reference's quirky reshape):
        q_h = M_q[h*64:(h+1)*64, :].reshape(N, 64)   (same for k_h, v_h)
        o[:, h*64:(h+1)*64] = softmax(q_h @ k_h.T) @ v_h
     out_b = o @ Wp + bp

Device kernel (one batch per NeuronCore, pure data-parallel SPMD, no
collectives): everything in bf16 matmuls with fp32 PSUM accumulation.
Queries and keys are processed in a permuted order nt = s*64 + r
(original n = r*16 + s) which turns the quirky head reshape into
hardware-friendly strided DMAs; the output DMA un-permutes rows.

Softmax has no max-subtraction (scores here are mathematically tiny).
Row sums come from an all-ones matmul on the tensor engine, which also
broadcasts them across all psum partitions; the division is applied to
the (transposed) attention output after a bf16 reciprocal.
"""

import os
import sys

import numpy as np

for _p in ("/opt/trn_rl_repo", "/root/.axon_site/_ro/trn_rl_repo"):
    if os.path.isdir(_p) and _p not in sys.path:
        sys.path.append(_p)

import ml_dtypes

import concourse.bass as bass
import concourse.mybir as mybir
import concourse.tile as tile
from concourse.bass_utils import run_bass_kernel_spmd

BF16 = ml_dtypes.bfloat16
B, N, C, H = 8, 1024, 1024, 16
HD = C // H            # 64
F = C // 2 + 1         # 513
NCORES = 8

# ---------------------------------------------------------------------------
# Walrus workaround: the staged neuronxcc rejects CTRL_NO_STRUCT instructions
# (the Tile kernel-tail Drain) carrying more than one SyncWait. Split excess
# waits onto dedicated no-fuse InstNoOp carriers on the same engine queue.
# ---------------------------------------------------------------------------
_MAX_WAITS = 1


def _split_waits_in_module(nc):
    for f in nc.m.functions:
        for bb in f.blocks:
            out, changed = [], False
            for inst in list(bb.instructions):
                si = inst.sync_info
                if si is not None and len(si.on_wait) > _MAX_WAITS:
                    waits = list(si.on_wait)
                    keep, excess = waits[-_MAX_WAITS:], waits[:-_MAX_WAITS]
                    for i in range(0, len(excess), _MAX_WAITS):
                        nop = mybir.InstNoOp(
                            name=f"I-{nc.next_id()}-waitcarrier",
                            engine=inst.engine,
                            bass_nofuse=True,
                            sync_info=mybir.SyncInfo(
                                on_wait=excess[i : i + _MAX_WAITS], on_update=[]
                            ),
                        )
                        nc.register_instruction(nop, overwrite=True)
                        out.append(nop)
                        changed = True
                    inst.sync_info = mybir.SyncInfo(
                        on_wait=keep, on_update=list(si.on_update)
                    )
                out.append(inst)
            if changed:
                bb.instructions = out


_orig_drain_and_barrier = tile.TileContext._drain_and_barrier


def _patched_drain_and_barrier(self, tick_clock, wait_clock):
    _orig_drain_and_barrier(self, tick_clock, wait_clock)
    _split_waits_in_module(self.nc)


def _install_tile_patch():
    tile.TileContext._drain_and_barrier = _patched_drain_and_barrier


_install_tile_patch()

# ---------------------------------------------------------------------------
# Host-side weight folding (DFT matrices are input-independent constants).
# ---------------------------------------------------------------------------


def _dft_matrices():
    c = np.arange(C)[:, None].astype(np.float64)
    j = np.arange(F)[None, :].astype(np.float64)
    ang = 2.0 * np.pi * c * j / C
    E = np.concatenate([np.cos(ang) / C, np.sin(ang) / C], axis=1)  # [C, 2F]
    Fh = C // 2
    jj = np.arange(Fh)[:, None].astype(np.float64)
    cc = np.arange(C)[None, :].astype(np.float64)
    ang2 = 2.0 * np.pi * jj * cc / C
    w = np.full((Fh, 1), 2.0)
    w[0, 0] = 1.0
    D = np.concatenate([w * np.cos(ang2), w * np.sin(ang2)], axis=0)  # [C, C]
    return E.astype(np.float32), D.astype(np.float32)


_E, _D = _dft_matrices()

# ---------------------------------------------------------------------------
# Device kernel builder.
# ---------------------------------------------------------------------------

F32 = mybir.dt.float32
BF = mybir.dt.bfloat16


def build_nc():
    nc = bass.Bass()
    xT = nc.declare_dram_parameter("xT", [C, N], BF, isOutput=False)
    wq = nc.declare_dram_parameter("wq", [C, C], BF, isOutput=False)
    wk = nc.declare_dram_parameter("wk", [C, C], BF, isOutput=False)
    wv = nc.declare_dram_parameter("wv", [C, C], BF, isOutput=False)
    wp = nc.declare_dram_parameter("wp", [C, C], BF, isOutput=False)
    bp = nc.declare_dram_parameter("bp", [1, C], BF, isOutput=False)
    out = nc.declare_dram_parameter("out", [N, C], F32, isOutput=True)

    # Intermediates, one tensor per head-pair so attention for a pair can
    # start as soon as its own projection columns land (Tile DRAM deps are
    # whole-tensor): qt/kt hold M_q.T / M_k.T column blocks [C, 128]; vv
    # holds M_v row blocks [128, C].
    qts = [nc.dram_tensor(f"qt{hp}", [C, 128], BF) for hp in range(8)]
    kts = [nc.dram_tensor(f"kt{hp}", [C, 128], BF) for hp in range(8)]
    vvs = [nc.dram_tensor(f"vv{mi}", [128, C], BF) for mi in range(8)]

    Exp = mybir.ActivationFunctionType.Exp
    Copy = mybir.ActivationFunctionType.Copy

    with tile.TileContext(nc) as tc:
        with (
            tc.tile_pool(name="const", bufs=1) as consts,
            tc.tile_pool(name="win", bufs=1) as win,
            tc.tile_pool(name="winx", bufs=1) as winx,
            tc.tile_pool(name="evict", bufs=4) as evict,
            tc.tile_pool(name="attin", bufs=2) as attin,
            tc.tile_pool(name="ptiles", bufs=2) as ptiles,
            tc.tile_pool(name="stiles", bufs=2) as stiles,
            tc.tile_pool(name="otiles", bufs=1) as otiles,
        ):
            # ---- load inputs into SBUF ----
            def load_rows(src, n_tiles=8, width=None, dtype=BF, pool=win,
                          eng=None):
                w_ = width or src.shape[1]
                ts = []
                for k in range(n_tiles):
                    t = pool.tile([128, w_], dtype, tag=f"ld_{src.tensor.name}_{k}")
                    (eng or nc.sync).dma_start(t[:], src[k * 128 : (k + 1) * 128, :])
                    ts.append(t)
                return ts

            # interleave x/wq so the first projection's deps land first
            xts, wqs = [], []
            for k in range(8):
                t = winx.tile([128, N], BF, tag=f"ld_xT_{k}", name="xt")
                (nc.sync if k % 2 == 0 else nc.gpsimd).dma_start(
                    t[:], xT[k * 128 : (k + 1) * 128, :]
                )
                xts.append(t)
                t = winx.tile([128, C], BF, tag=f"ld_wq_{k}", name="wqt")
                nc.scalar.dma_start(t[:], wq[k * 128 : (k + 1) * 128, :])
                wqs.append(t)
            wks = load_rows(wk[:], pool=winx)
            wvs = load_rows(wv[:], pool=winx, eng=nc.gpsimd)
            wps = load_rows(wp[:], eng=nc.gpsimd)

            bpt = consts.tile([1, C], BF)
            nc.sync.dma_start(bpt[:], bp[:])
            ones_row = consts.tile([1, 128], BF)
            nc.gpsimd.memset(ones_row[:], 1.0)
            ones128 = consts.tile([128, 128], BF)
            nc.gpsimd.memset(ones128[:], 1.0)

            psum_ctx = tc.tile_pool(name="psatt", bufs=2, space="PSUM")
            psum = psum_ctx.__enter__()

            # ---- phase 1: projections, emitted per head-pair ----
            # qt_hp = (Wq.T @ xT)[:, hp*128:+128]  as [C, 128]  (N=128 tiles)
            # kt_hp likewise; vv_hp = (x @ Wv)[hp*128:+128, :]  as [128, C]
            def proj_pair(hp):
                for wt, dst in ((wqs, qts[hp]), (wks, kts[hp])):
                    for mi in range(8):
                        ps = psum.tile([128, 128], F32, tag="proj", name="ps",
                                       bufs=1)
                        for ki in range(8):
                            nc.tensor.matmul(
                                ps[:],
                                wt[ki][:, mi * 128 : (mi + 1) * 128],
                                xts[ki][:, hp * 128 : (hp + 1) * 128],
                                start=(ki == 0), stop=(ki == 7),
                            )
                        sb = evict.tile([128, 128], BF, tag="projev", name="sb")
                        nc.vector.tensor_copy(sb[:], ps[:])
                        nc.sync.dma_start(
                            dst[mi * 128 : (mi + 1) * 128, :], sb[:]
                        )
                for ni in range(2):
                    ps = psum.tile([128, 512], F32, tag="proj", name="ps",
                                   bufs=1)
                    for ki in range(8):
                        nc.tensor.matmul(
                            ps[:],
                            xts[ki][:, hp * 128 : (hp + 1) * 128],
                            wvs[ki][:, ni * 512 : (ni + 1) * 512],
                            start=(ki == 0), stop=(ki == 7),
                        )
                    sb = evict.tile([128, 512], BF, tag="projev", name="sb")
                    nc.scalar.activation(sb[:], ps[:], Copy)
                    nc.sync.dma_start(
                        vvs[hp][:, ni * 512 : (ni + 1) * 512], sb[:]
                    )

            oT_tiles = []

            # ---- phase 2: attention, one head-pair at a time ----
            def attention_pair(hp):
                qt_v = qts[hp][:].rearrange(
                    "(s d) (h r) -> h d s r", s=16, d=64, h=2, r=64
                )
                kt_v = kts[hp][:].rearrange(
                    "(s d) (h r) -> h d s r", s=16, d=64, h=2, r=64
                )
                vv_v = vvs[hp][:].rearrange(
                    "(h r) (sc s2 d) -> h s2 r sc d", h=2, r=64, sc=8, s2=2, d=64
                )
                qtp = attin.tile([128, N], BF, tag="qtp")
                ktp = attin.tile([128, N], BF, tag="ktp")
                vp = attin.tile([128, N], BF, tag="vp")
                for h in range(2):
                    hs = slice(h * 64, (h + 1) * 64)
                    nc.sync.dma_start(qtp[hs, :], qt_v[h])
                    nc.sync.dma_start(ktp[hs, :], kt_v[h])
                    for s2 in range(2):
                        vslice = vp[s2 * 64 : (s2 + 1) * 64, :].rearrange(
                            "p (sc h d) -> p sc h d", sc=8, h=2, d=64
                        )[:, :, h, :]
                        nc.gpsimd.dma_start(vslice, vv_v[h, s2])

                oTp = otiles.tile([128, N], BF, tag=f"oT{hp}")
                oT_tiles.append(oTp)

                for ni in range(2):
                    n1s = slice(ni * 512, (ni + 1) * 512)
                    PA = ptiles.tile([128, 8 * 512], BF, tag="PA")
                    PB = ptiles.tile([128, 8 * 512], BF, tag="PB")
                    # scores + exp: scoreT[nt2, nt1] row-tiled head pair.
                    # Two nt2-chunks share one 2-bank psum tile so each exp
                    # covers [128, 1024] (halves ACT per-op overhead).
                    for c2p in range(4):
                        psa = psum.tile([128, 1024], F32, tag="scA", bufs=1)
                        psb = psum.tile([128, 1024], F32, tag="scB", bufs=1)
                        for k in range(2):
                            c2 = 2 * c2p + k
                            c2s = slice(c2 * 128, (c2 + 1) * 128)
                            ph = slice(k * 512, (k + 1) * 512)
                            nc.tensor.matmul(
                                psa[:, ph], ktp[0:64, c2s], qtp[0:64, n1s],
                                start=True, stop=True, tile_position=(0, 0),
                            )
                            nc.tensor.matmul(
                                psb[:, ph], ktp[64:128, c2s], qtp[64:128, n1s],
                                start=True, stop=True, tile_position=(64, 0),
                            )
                        nc.scalar.activation(
                            PA[:, c2p * 1024 : (c2p + 1) * 1024], psa[:], Exp
                        )
                        nc.scalar.activation(
                            PB[:, c2p * 1024 : (c2p + 1) * 1024], psb[:], Exp
                        )
                    # row sums: chunk-reduce on DVE, then ones-matmul on PE,
                    # reciprocal, partition-broadcast
                    rbs = []
                    for P_, tagc in ((PA, "A"), (PB, "B")):
                        # pairwise add-tree over the 8 chunks (bf16 4x DVE
                        # mode); leaf adds start as soon as adjacent exp
                        # chunks land, shortening the tail chain
                        S2 = stiles.tile([128, 2048], BF, tag=f"S2{tagc}", bufs=1)
                        for q in range(4):
                            nc.vector.tensor_add(
                                S2[:, q * 512 : (q + 1) * 512],
                                P_[:, (2 * q) * 512 : (2 * q + 1) * 512],
                                P_[:, (2 * q + 1) * 512 : (2 * q + 2) * 512],
                            )
                        S4 = stiles.tile([128, 1024], BF, tag=f"S4{tagc}", bufs=1)
                        for q in range(2):
                            nc.vector.tensor_add(
                                S4[:, q * 512 : (q + 1) * 512],
                                S2[:, (2 * q) * 512 : (2 * q + 1) * 512],
                                S2[:, (2 * q + 1) * 512 : (2 * q + 2) * 512],
                            )
                        S = stiles.tile([128, 512], BF, tag=f"S{tagc}", bufs=1)
                        nc.vector.tensor_add(S[:], S4[:, 0:512], S4[:, 512:1024])
                        # all-ones lhsT: one matmul yields the row-sum
                        # already broadcast across all 128 psum partitions
                        rps = psum.tile([128, 512], F32, tag="psR", bufs=1,
                                        name="rps")
                        nc.tensor.matmul(
                            rps[:], ones128[:], S[:], start=True, stop=True
                        )
                        rb = stiles.tile([128, 512], BF, tag=f"rb{tagc}",
                                         bufs=1)
                        with nc.allow_low_precision(
                            reason="softmax 1/rowsum in bf16; fine vs the "
                            "2e-2 gate"
                        ):
                            nc.vector.reciprocal(rb[:], rps[:])
                        rbs.append(rb)
                    # o.T = v.T @ P, col-tiled head pair, accumulate over nt2
                    psoA = psum.tile([128, 512], F32, tag="psOA", bufs=1)
                    psoB = psum.tile([128, 512], F32, tag="psOB", bufs=1)
                    for c2 in range(8):
                        nc.tensor.matmul(
                            psoA[0:64, :],
                            vp[:, c2 * 128 : c2 * 128 + 64],
                            PA[:, c2 * 512 : (c2 + 1) * 512],
                            start=(c2 == 0), stop=(c2 == 7), tile_position=(0, 0),
                        )
                        nc.tensor.matmul(
                            psoB[64:128, :],
                            vp[:, c2 * 128 + 64 : c2 * 128 + 128],
                            PB[:, c2 * 512 : (c2 + 1) * 512],
                            start=(c2 == 0), stop=(c2 == 7), tile_position=(0, 64),
                        )
                    nc.vector.tensor_mul(oTp[0:64, n1s], psoA[0:64, :], rbs[0][0:64, :])
                    nc.vector.tensor_mul(
                        oTp[64:128, n1s], psoB[64:128, :], rbs[1][64:128, :]
                    )

            proj_pair(0)
            proj_pair(1)
            for hp in range(8):
                attention_pair(hp)
                if hp + 2 < 8:
                    proj_pair(hp + 2)

            # ---- phase 3: out = o @ Wp + bp, rows written un-permuted ----
            winx_released = True
            psum_ctx.__exit__(None, None, None)
            psfin_ctx = tc.tile_pool(name="psfin", bufs=4, space="PSUM")
            psfin = psfin_ctx.__enter__()
            out_v = out[:].rearrange("(r sm s2) c -> sm s2 r c", r=64, sm=8, s2=2)
            for mi in range(8):
                for ni in range(2):
                    ps = psfin.tile([128, 512], F32, tag="fin")
                    nc.tensor.matmul(
                        ps[:], ones_row[:], bpt[0:1, ni * 512 : (ni + 1) * 512],
                        start=True, stop=False,
                    )
                    for kp in range(8):
                        nc.tensor.matmul(
                            ps[:],
                            oT_tiles[kp][:, mi * 128 : (mi + 1) * 128],
                            wps[kp][:, ni * 512 : (ni + 1) * 512],
                            start=False, stop=(kp == 7),
                        )
                    ob = evict.tile([128, 512], F32, tag="outev")
                    nc.scalar.activation(ob[:], ps[:], Copy)
                    for s2 in range(2):
                        nc.sync.dma_start(
                            out_v[mi, s2][:, ni * 512 : (ni + 1) * 512],
                            ob[s2 * 64 : (s2 + 1) * 64, :],
                        )
            psfin_ctx.__exit__(None, None, None)

    return nc


# ---------------------------------------------------------------------------
# Host wrapper.
# ---------------------------------------------------------------------------

_NC_CACHE = None


def _get_nc():
    global _NC_CACHE
    if _NC_CACHE is None:
        _NC_CACHE = build_nc()
    return _NC_CACHE


def host_inputs(x, wq, wk, wv, wproj, bproj):
    """Fold DFT matrices into the weights; per-core input maps."""
    scale = float(HD) ** -0.5
    Wq = (_E @ wq.T.astype(np.float32) * scale).astype(BF16)
    Wk = (_E @ wk.T.astype(np.float32)).astype(BF16)
    Wv = (_E @ wv.T.astype(np.float32)).astype(BF16)
    Wp = (wproj.T.astype(np.float32) @ _D).astype(BF16)
    bpD = (bproj.astype(np.float32) @ _D).astype(BF16).reshape(1, C)
    in_maps = []
    for b in range(B):
        xTb = np.ascontiguousarray(x[b].T).astype(BF16)
        in_maps.append(
            {"xT": xTb, "wq": Wq, "wk": Wk, "wv": Wv, "wp": Wp, "bp": bpD}
        )
    return in_maps


def kernel(x, wq, wk, wv, wproj, bproj):
    x = np.asarray(x, dtype=np.float32)
    in_maps = host_inputs(
        x,
        np.asarray(wq, np.float32),
        np.asarray(wk, np.float32),
        np.asarray(wv, np.float32),
        np.asarray(wproj, np.float32),
        np.asarray(bproj, np.float32),
    )
    nc = _get_nc()
    res = run_bass_kernel_spmd(nc, in_maps, list(range(NCORES)))
    out = np.stack([np.asarray(res.results[i]["out"]) for i in range(NCORES)])
    return out.astype(np.float32)


# revision 58
# speedup vs baseline: 1.0927x; 1.0007x over previous
"""FFT-encoded attention (nn_Attention_78065325572136) on 8 Trainium2 cores.

Math (per batch b, reproducing the reference exactly):
  feat = [Re rfft(x)/C, -Im rfft(x)/C]  ->  folded into weights on host:
     Wq = E @ wq.T * hd^-0.5,  Wk = E @ wk.T,  Wv = E @ wv.T   (E = DFT-real matrix)
     Wp = wproj.T @ D,  bp = bproj @ D                         (D = irfft matrix)
  so the whole module becomes matmuls + softmax:
     M_q = x_b @ Wq, M_k = x_b @ Wk, M_v = x_b @ Wv            [N, C]
     per head h (the reference's quirky reshape):
        q_h = M_q[h*64:(h+1)*64, :].reshape(N, 64)   (same for k_h, v_h)
        o[:, h*64:(h+1)*64] = softmax(q_h @ k_h.T) @ v_h
     out_b = o @ Wp + bp

Device kernel (one batch per NeuronCore, pure data-parallel SPMD, no
collectives): everything in bf16 matmuls with fp32 PSUM accumulation.
Queries and keys are processed in a permuted order nt = s*64 + r
(original n = r*16 + s) which turns the quirky head reshape into
hardware-friendly strided DMAs; the output DMA un-permutes rows.

Softmax has no max-subtraction (scores here are mathematically tiny).
Row sums come from an all-ones matmul on the tensor engine, which also
broadcasts them across all psum partitions; the division is applied to
the (transposed) attention output after a bf16 reciprocal.
"""

import os
import sys

import numpy as np

for _p in ("/opt/trn_rl_repo", "/root/.axon_site/_ro/trn_rl_repo"):
    if os.path.isdir(_p) and _p not in sys.path:
        sys.path.append(_p)

import ml_dtypes

import concourse.bass as bass
import concourse.mybir as mybir
import concourse.tile as tile
from concourse.bass_utils import run_bass_kernel_spmd

BF16 = ml_dtypes.bfloat16
B, N, C, H = 8, 1024, 1024, 16
HD = C // H            # 64
F = C // 2 + 1         # 513
NCORES = 8

# ---------------------------------------------------------------------------
# Walrus workaround: the staged neuronxcc rejects CTRL_NO_STRUCT instructions
# (the Tile kernel-tail Drain) carrying more than one SyncWait. Split excess
# waits onto dedicated no-fuse InstNoOp carriers on the same engine queue.
# ---------------------------------------------------------------------------
_MAX_WAITS = 1


def _split_waits_in_module(nc):
    for f in nc.m.functions:
        for bb in f.blocks:
            out, changed = [], False
            for inst in list(bb.instructions):
                si = inst.sync_info
                if si is not None and len(si.on_wait) > _MAX_WAITS:
                    waits = list(si.on_wait)
                    keep, excess = waits[-_MAX_WAITS:], waits[:-_MAX_WAITS]
                    for i in range(0, len(excess), _MAX_WAITS):
                        nop = mybir.InstNoOp(
                            name=f"I-{nc.next_id()}-waitcarrier",
                            engine=inst.engine,
                            bass_nofuse=True,
                            sync_info=mybir.SyncInfo(
                                on_wait=excess[i : i + _MAX_WAITS], on_update=[]
                            ),
                        )
                        nc.register_instruction(nop, overwrite=True)
                        out.append(nop)
                        changed = True
                    inst.sync_info = mybir.SyncInfo(
                        on_wait=keep, on_update=list(si.on_update)
                    )
                out.append(inst)
            if changed:
                bb.instructions = out


_orig_drain_and_barrier = tile.TileContext._drain_and_barrier


def _patched_drain_and_barrier(self, tick_clock, wait_clock):
    _orig_drain_and_barrier(self, tick_clock, wait_clock)
    _split_waits_in_module(self.nc)


def _install_tile_patch():
    tile.TileContext._drain_and_barrier = _patched_drain_and_barrier


_install_tile_patch()

# ---------------------------------------------------------------------------
# Host-side weight folding (DFT matrices are input-independent constants).
# ---------------------------------------------------------------------------


def _dft_matrices():
    c = np.arange(C)[:, None].astype(np.float64)
    j = np.arange(F)[None, :].astype(np.float64)
    ang = 2.0 * np.pi * c * j / C
    E = np.concatenate([np.cos(ang) / C, np.sin(ang) / C], axis=1)  # [C, 2F]
    Fh = C // 2
    jj = np.arange(Fh)[:, None].astype(np.float64)
    cc = np.arange(C)[None, :].astype(np.float64)
    ang2 = 2.0 * np.pi * jj * cc / C
    w = np.full((Fh, 1), 2.0)
    w[0, 0] = 1.0
    D = np.concatenate([w * np.cos(ang2), w * np.sin(ang2)], axis=0)  # [C, C]
    return E.astype(np.float32), D.astype(np.float32)


_E, _D = _dft_matrices()

# ---------------------------------------------------------------------------
# Device kernel builder.
# ---------------------------------------------------------------------------

F32 = mybir.dt.float32
BF = mybir.dt.bfloat16


def build_nc():
    nc = bass.Bass()
    xT = nc.declare_dram_parameter("xT", [C, N], BF, isOutput=False)
    wq = nc.declare_dram_parameter("wq", [C, C], BF, isOutput=False)
    wk = nc.declare_dram_parameter("wk", [C, C], BF, isOutput=False)
    wv = nc.declare_dram_parameter("wv", [C, C], BF, isOutput=False)
    wp = nc.declare_dram_parameter("wp", [C, C], BF, isOutput=False)
    bp = nc.declare_dram_parameter("bp", [1, C], BF, isOutput=False)
    out = nc.declare_dram_parameter("out", [N, C], F32, isOutput=True)

    # Intermediates, one tensor per head-pair so attention for a pair can
    # start as soon as its own projection columns land (Tile DRAM deps are
    # whole-tensor): qt/kt hold M_q.T / M_k.T column blocks [C, 128]; vv
    # holds M_v row blocks [128, C].
    qts = [nc.dram_tensor(f"qt{hp}", [C, 128], BF) for hp in range(8)]
    kts = [nc.dram_tensor(f"kt{hp}", [C, 128], BF) for hp in range(8)]
    vvs = [nc.dram_tensor(f"vv{mi}", [128, C], BF) for mi in range(8)]

    Exp = mybir.ActivationFunctionType.Exp
    Copy = mybir.ActivationFunctionType.Copy

    with tile.TileContext(nc) as tc:
        with (
            tc.tile_pool(name="const", bufs=1) as consts,
            tc.tile_pool(name="win", bufs=1) as win,
            tc.tile_pool(name="winx", bufs=1) as winx,
            tc.tile_pool(name="evict", bufs=4) as evict,
            tc.tile_pool(name="attin", bufs=2) as attin,
            tc.tile_pool(name="ptiles", bufs=2) as ptiles,
            tc.tile_pool(name="stiles", bufs=2) as stiles,
            tc.tile_pool(name="otiles", bufs=1) as otiles,
        ):
            # ---- load inputs into SBUF ----
            def load_rows(src, n_tiles=8, width=None, dtype=BF, pool=win,
                          eng=None):
                w_ = width or src.shape[1]
                ts = []
                for k in range(n_tiles):
                    t = pool.tile([128, w_], dtype, tag=f"ld_{src.tensor.name}_{k}")
                    (eng or nc.sync).dma_start(t[:], src[k * 128 : (k + 1) * 128, :])
                    ts.append(t)
                return ts

            # interleave x/wq/wk so the first projections' deps land first
            xts, wqs, wks = [], [], []
            for k in range(8):
                t = winx.tile([128, N], BF, tag=f"ld_xT_{k}", name="xt")
                (nc.sync if k % 2 == 0 else nc.gpsimd).dma_start(
                    t[:], xT[k * 128 : (k + 1) * 128, :]
                )
                xts.append(t)
                t = winx.tile([128, C], BF, tag=f"ld_wq_{k}", name="wqt")
                nc.scalar.dma_start(t[:], wq[k * 128 : (k + 1) * 128, :])
                wqs.append(t)
                t = winx.tile([128, C], BF, tag=f"ld_wk_{k}", name="wkt")
                nc.sync.dma_start(t[:], wk[k * 128 : (k + 1) * 128, :])
                wks.append(t)
            wvs = load_rows(wv[:], pool=winx, eng=nc.gpsimd)
            wps = load_rows(wp[:], eng=nc.gpsimd)

            bpt = consts.tile([1, C], BF)
            nc.sync.dma_start(bpt[:], bp[:])
            ones_row = consts.tile([1, 128], BF)
            nc.gpsimd.memset(ones_row[:], 1.0)
            ones128 = consts.tile([128, 128], BF)
            nc.gpsimd.memset(ones128[:], 1.0)

            psum_ctx = tc.tile_pool(name="psatt", bufs=2, space="PSUM")
            psum = psum_ctx.__enter__()

            # ---- phase 1: projections, emitted per head-pair ----
            # qt_hp = (Wq.T @ xT)[:, hp*128:+128]  as [C, 128]  (N=128 tiles)
            # kt_hp likewise; vv_hp = (x @ Wv)[hp*128:+128, :]  as [128, C]
            def proj_pair(hp):
                for wt, dst in ((wqs, qts[hp]), (wks, kts[hp])):
                    for mi in range(8):
                        ps = psum.tile([128, 128], F32, tag="proj", name="ps",
                                       bufs=1)
                        for ki in range(8):
                            nc.tensor.matmul(
                                ps[:],
                                wt[ki][:, mi * 128 : (mi + 1) * 128],
                                xts[ki][:, hp * 128 : (hp + 1) * 128],
                                start=(ki == 0), stop=(ki == 7),
                            )
                        sb = evict.tile([128, 128], BF, tag="projev", name="sb")
                        nc.vector.tensor_copy(sb[:], ps[:])
                        nc.sync.dma_start(
                            dst[mi * 128 : (mi + 1) * 128, :], sb[:]
                        )
                for ni in range(2):
                    ps = psum.tile([128, 512], F32, tag="proj", name="ps",
                                   bufs=1)
                    for ki in range(8):
                        nc.tensor.matmul(
                            ps[:],
                            xts[ki][:, hp * 128 : (hp + 1) * 128],
                            wvs[ki][:, ni * 512 : (ni + 1) * 512],
                            start=(ki == 0), stop=(ki == 7),
                        )
                    sb = evict.tile([128, 512], BF, tag="projev", name="sb")
                    nc.scalar.activation(sb[:], ps[:], Copy)
                    nc.sync.dma_start(
                        vvs[hp][:, ni * 512 : (ni + 1) * 512], sb[:]
                    )

            oT_tiles = []

            # ---- phase 2: attention, one head-pair at a time ----
            def attention_pair(hp):
                qt_v = qts[hp][:].rearrange(
                    "(s d) (h r) -> h d s r", s=16, d=64, h=2, r=64
                )
                kt_v = kts[hp][:].rearrange(
                    "(s d) (h r) -> h d s r", s=16, d=64, h=2, r=64
                )
                vv_v = vvs[hp][:].rearrange(
                    "(h r) (sc s2 d) -> h s2 r sc d", h=2, r=64, sc=8, s2=2, d=64
                )
                qtp = attin.tile([128, N], BF, tag="qtp")
                ktp = attin.tile([128, N], BF, tag="ktp")
                vp = attin.tile([128, N], BF, tag="vp")
                for h in range(2):
                    hs = slice(h * 64, (h + 1) * 64)
                    nc.sync.dma_start(qtp[hs, :], qt_v[h])
                    nc.sync.dma_start(ktp[hs, :], kt_v[h])
                    for s2 in range(2):
                        vslice = vp[s2 * 64 : (s2 + 1) * 64, :].rearrange(
                            "p (sc h d) -> p sc h d", sc=8, h=2, d=64
                        )[:, :, h, :]
                        nc.gpsimd.dma_start(vslice, vv_v[h, s2])

                oTp = otiles.tile([128, N], BF, tag=f"oT{hp}")
                oT_tiles.append(oTp)

                for ni in range(2):
                    n1s = slice(ni * 512, (ni + 1) * 512)
                    PA = ptiles.tile([128, 8 * 512], BF, tag="PA")
                    PB = ptiles.tile([128, 8 * 512], BF, tag="PB")
                    # scores + exp: scoreT[nt2, nt1] row-tiled head pair.
                    # Two nt2-chunks share one 2-bank psum tile so each exp
                    # covers [128, 1024] (halves ACT per-op overhead).
                    for c2p in range(4):
                        psa = psum.tile([128, 1024], F32, tag="scA", bufs=1)
                        psb = psum.tile([128, 1024], F32, tag="scB", bufs=1)
                        for k in range(2):
                            c2 = 2 * c2p + k
                            c2s = slice(c2 * 128, (c2 + 1) * 128)
                            ph = slice(k * 512, (k + 1) * 512)
                            nc.tensor.matmul(
                                psa[:, ph], ktp[0:64, c2s], qtp[0:64, n1s],
                                start=True, stop=True, tile_position=(0, 0),
                            )
                            nc.tensor.matmul(
                                psb[:, ph], ktp[64:128, c2s], qtp[64:128, n1s],
                                start=True, stop=True, tile_position=(64, 0),
                            )
                        nc.scalar.activation(
                            PA[:, c2p * 1024 : (c2p + 1) * 1024], psa[:], Exp
                        )
                        nc.scalar.activation(
                            PB[:, c2p * 1024 : (c2p + 1) * 1024], psb[:], Exp
                        )
                    # row sums: chunk-reduce on DVE, then ones-matmul on PE,
                    # reciprocal, partition-broadcast
                    rbs = []
                    for P_, tagc in ((PA, "A"), (PB, "B")):
                        # pairwise add-tree over the 8 chunks (bf16 4x DVE
                        # mode); leaf adds start as soon as adjacent exp
                        # chunks land, shortening the tail chain
                        S2 = stiles.tile([128, 2048], BF, tag=f"S2{tagc}", bufs=1)
                        for q in range(4):
                            nc.vector.tensor_add(
                                S2[:, q * 512 : (q + 1) * 512],
                                P_[:, (2 * q) * 512 : (2 * q + 1) * 512],
                                P_[:, (2 * q + 1) * 512 : (2 * q + 2) * 512],
                            )
                        S4 = stiles.tile([128, 1024], BF, tag=f"S4{tagc}", bufs=1)
                        for q in range(2):
                            nc.vector.tensor_add(
                                S4[:, q * 512 : (q + 1) * 512],
                                S2[:, (2 * q) * 512 : (2 * q + 1) * 512],
                                S2[:, (2 * q + 1) * 512 : (2 * q + 2) * 512],
                            )
                        S = stiles.tile([128, 512], BF, tag=f"S{tagc}", bufs=1)
                        nc.vector.tensor_add(S[:], S4[:, 0:512], S4[:, 512:1024])
                        # all-ones lhsT: one matmul yields the row-sum
                        # already broadcast across all 128 psum partitions
                        rps = psum.tile([128, 512], F32, tag="psR", bufs=1,
                                        name="rps")
                        nc.tensor.matmul(
                            rps[:], ones128[:], S[:], start=True, stop=True
                        )
                        rb = stiles.tile([128, 512], BF, tag=f"rb{tagc}",
                                         bufs=1)
                        with nc.allow_low_precision(
                            reason="softmax 1/rowsum in bf16; fine vs the "
                            "2e-2 gate"
                        ):
                            nc.vector.reciprocal(rb[:], rps[:])
                        rbs.append(rb)
                    # o.T = v.T @ P, col-tiled head pair, accumulate over nt2
                    psoA = psum.tile([128, 512], F32, tag="psOA", bufs=1)
                    psoB = psum.tile([128, 512], F32, tag="psOB", bufs=1)
                    for c2 in range(8):
                        nc.tensor.matmul(
                            psoA[0:64, :],
                            vp[:, c2 * 128 : c2 * 128 + 64],
                            PA[:, c2 * 512 : (c2 + 1) * 512],
                            start=(c2 == 0), stop=(c2 == 7), tile_position=(0, 0),
                        )
                        nc.tensor.matmul(
                            psoB[64:128, :],
                            vp[:, c2 * 128 + 64 : c2 * 128 + 128],
                            PB[:, c2 * 512 : (c2 + 1) * 512],
                            start=(c2 == 0), stop=(c2 == 7), tile_position=(0, 64),
                        )
                    nc.vector.tensor_mul(oTp[0:64, n1s], psoA[0:64, :], rbs[0][0:64, :])
                    nc.vector.tensor_mul(
                        oTp[64:128, n1s], psoB[64:128, :], rbs[1][64:128, :]
                    )

            proj_pair(0)
            proj_pair(1)
            for hp in range(8):
                attention_pair(hp)
                if hp + 2 < 8:
                    proj_pair(hp + 2)

            # ---- phase 3: out = o @ Wp + bp, rows written un-permuted ----
            winx_released = True
            psum_ctx.__exit__(None, None, None)
            psfin_ctx = tc.tile_pool(name="psfin", bufs=4, space="PSUM")
            psfin = psfin_ctx.__enter__()
            out_v = out[:].rearrange("(r sm s2) c -> sm s2 r c", r=64, sm=8, s2=2)
            for mi in range(8):
                for ni in range(2):
                    ps = psfin.tile([128, 512], F32, tag="fin")
                    nc.tensor.matmul(
                        ps[:], ones_row[:], bpt[0:1, ni * 512 : (ni + 1) * 512],
                        start=True, stop=False,
                    )
                    for kp in range(8):
                        nc.tensor.matmul(
                            ps[:],
                            oT_tiles[kp][:, mi * 128 : (mi + 1) * 128],
                            wps[kp][:, ni * 512 : (ni + 1) * 512],
                            start=False, stop=(kp == 7),
                        )
                    ob = evict.tile([128, 512], F32, tag="outev")
                    nc.scalar.activation(ob[:], ps[:], Copy)
                    for s2 in range(2):
                        nc.sync.dma_start(
                            out_v[mi, s2][:, ni * 512 : (ni + 1) * 512],
                            ob[s2 * 64 : (s2 + 1) * 64, :],
                        )
            psfin_ctx.__exit__(None, None, None)

    return nc


# ---------------------------------------------------------------------------
# Host wrapper.
# ---------------------------------------------------------------------------

_NC_CACHE = None


def _get_nc():
    global _NC_CACHE
    if _NC_CACHE is None:
        _NC_CACHE = build_nc()
    return _NC_CACHE


def host_inputs(x, wq, wk, wv, wproj, bproj):
    """Fold DFT matrices into the weights; per-core input maps."""
    scale = float(HD) ** -0.5
    Wq = (_E @ wq.T.astype(np.float32) * scale).astype(BF16)
    Wk = (_E @ wk.T.astype(np.float32)).astype(BF16)
    Wv = (_E @ wv.T.astype(np.float32)).astype(BF16)
    Wp = (wproj.T.astype(np.float32) @ _D).astype(BF16)
    bpD = (bproj.astype(np.float32) @ _D).astype(BF16).reshape(1, C)
    in_maps = []
    for b in range(B):
        xTb = np.ascontiguousarray(x[b].T).astype(BF16)
        in_maps.append(
            {"xT": xTb, "wq": Wq, "wk": Wk, "wv": Wv, "wp": Wp, "bp": bpD}
        )
    return in_maps


def kernel(x, wq, wk, wv, wproj, bproj):
    x = np.asarray(x, dtype=np.float32)
    in_maps = host_inputs(
        x,
        np.asarray(wq, np.float32),
        np.asarray(wk, np.float32),
        np.asarray(wv, np.float32),
        np.asarray(wproj, np.float32),
        np.asarray(bproj, np.float32),
    )
    nc = _get_nc()
    res = run_bass_kernel_spmd(nc, in_maps, list(range(NCORES)))
    out = np.stack([np.asarray(res.results[i]["out"]) for i in range(NCORES)])
    return out.astype(np.float32)
